# revision 19
# baseline (speedup 1.0000x reference)
"""BERT self-attention (B=4, S=2048, H=768, 12 heads x d=64) on 8 Trainium2
NeuronCores.

Sharding: core c handles batch b = c//2 and head group hg = c%2 (6 heads).
No cross-core communication; the host scatters inputs and gathers the output.

Per-core pipeline, per 512-query window (chunk = (head pair, i-chunk)):
  projections  x^T bf16 @ W bf16 -> q/k planes [65, S] per head (row 64 is a
               bias row injecting +16192 = 126.5*128 into every score), v
               bf16 with 64 ones-columns per head (sumexp rides the ctx
               matmul into psum rows 64-127)
  scores       per head: K=65 matmul -> psum z = 128*log2e*logit + 16192
  exp          split across engines, both writing bf16 probs:
                 Act: exp(z*ln2/128 - 126.5*ln2)
                 DVE: custom op EXP_BITS_ANT: n = z + K + c*(u+g)^2 with
                 u = z - rtne_128(z), int16(n) bitcast bf16 == e^logit
                 (+-0.7%); the +16192 bias makes rtne a floor and funds the
                 16256 bf16 exponent-bias inside z, fitting the DVE's 8-stage
                 pipeline and 4 scalar slots
  ctx          bf16 matmuls accumulating over j-tiles; rows 0-63 ctx^T,
               rows 64-127 sumexp copies
  finalize     DVE: copy sumexp rows down to partitions 0-63,
               reciprocal_approx_fast, one multiply; DMA out

Emission is software-pipelined at j-tile granularity: scores+exp of chunk
w interleave with ctx of chunk w-1 on the PE queue.
"""
import os

import numpy as np

if not os.environ.get("KERNEL_TRACE"):
    os.environ.setdefault("BASS_NEVER_TRACE", "1")

import concourse.bass as bass
import concourse.dve_ops as dve_ops
import concourse.mybir as mybir
import concourse.tile as tile
from concourse import bacc
from concourse.bass import ts
from concourse.bass_utils import run_bass_kernel_spmd
from concourse.dve_spec import C0, C1, C2, C3, Spec, Src0, _spill_c3_to_src1, sq
from concourse.dve_spec import lower as dve_lower
from concourse.dve_uop import DveOpSpec

F32 = mybir.dt.float32
BF16 = mybir.dt.bfloat16
I16 = mybir.dt.int16

HIDDEN = 768
N_HEADS = 12
HEAD_DIM = 64
B = 4
S = 2048
HPC = 6          # heads per core
KC = 7           # contraction chunks of 128 (768 data + bias row + pad)
NI = S // 512    # 4 i-chunks of 512
NJ = S // 128    # 16 j-tiles of 128
VW = 128         # v (64) | ones (64): sumexp lands in psum rows 64-127

# z = 128*log2e*logit + QROW*KROW; QROW*KROW = 16192 = 16256 - 64 folds the
# bf16 exponent bias (127*128) minus the half-quantum floor shift into z.
QROW = 126.5
KROW = 128.0
ZBIAS = QROW * KROW
ACT_SCALE = float(np.log(2.0) / 128.0)
ACT_BIAS = float(-(ZBIAS / 128.0) * np.log(2.0))
MAGIC = float(1.5 * 2 ** 30)
EXP_C1 = -1.2418666827660498            # g (complete-the-square shift)
EXP_C2 = 0.002682149753064359           # c (quadratic bow correction)
EXP_C3 = 16308.998445149862 - 16256.0   # K'' additive constant
SCH_A = 128.0 * float(np.log2(np.e))    # mask -> z units

# exp engine assignment per j-tile: 10 Act / 6 DVE
ENG = ["a", "d", "a", "d", "a", "d", "a", "a",
       "d", "a", "d", "a", "a", "d", "a", "a"]

_cache = {}
last_results = None


def _exp_bits_ref(in0, in1, s0, s1, imm2):
    P = (in0.astype(np.float32) + np.float32(s0)).astype(np.float32)
    R = (P - np.float32(s0)).astype(np.float32)
    u = (in0 - R).astype(np.float32)
    t = (u + np.float32(s1)).astype(np.float32)
    t = (t * t).astype(np.float32)
    t = (t * np.float32(imm2)).astype(np.float32)
    t = (t + in0).astype(np.float32)
    return (t + in1.astype(np.float32)).astype(np.float32)


def _register_exp_bits():
    for op in dve_ops.OPS:
        if op.name == "EXP_BITS_ANT":
            return op
    u = Src0 - ((Src0 + C0) - C0)
    body = _spill_c3_to_src1(((sq(u + C1) * C2) + Src0) + C3)
    spec = Spec(body=body, reference=_exp_bits_ref)
    opcode = max(dve_ops._SUB_OPCODE_FOR_NAME.values()) + 1
    shas = {}
    for ver in ("v3", "v4"):
        s = DveOpSpec(name="EXP_BITS_ANT", opcode=opcode,
                      uops=dve_lower(spec, ver=ver), rd1_en=True)
        shas[ver] = s.sha(ver)
    op = dve_ops.DveOp("EXP_BITS_ANT", spec, subdim=False, uops_sha=shas)
    dve_ops.OPS.append(op)
    dve_ops.CUSTOM_DVE_SPECS["EXP_BITS_ANT"] = op.spec
    dve_ops._SUB_OPCODE_FOR_NAME["EXP_BITS_ANT"] = opcode
    return op


EXP_BITS = _register_exp_bits()


def _build(use_mask: bool, use_bias: bool):
    nc = bacc.Bacc("TRN2", target_bir_lowering=False, debug=False, num_devices=8)

    xT_d = nc.dram_tensor("xT", [KC * 128, S], BF16, kind="ExternalInput")
    wq_d = nc.dram_tensor("wq", [KC * 128, HPC * HEAD_DIM], BF16, kind="ExternalInput")
    wk_d = nc.dram_tensor("wk", [KC * 128, HPC * HEAD_DIM], BF16, kind="ExternalInput")
    wv_d = nc.dram_tensor("wv", [KC * 128, HPC * HEAD_DIM], BF16, kind="ExternalInput")
    if use_mask:
        # per-key bias row: (ZBIAS + 128*log2e*mask_j) / QROW
        kb_d = nc.dram_tensor("kb", [1, S], BF16, kind="ExternalInput")
    out_d = nc.dram_tensor("out", [HPC, HEAD_DIM, S], F32, kind="ExternalOutput")

    with tile.TileContext(nc) as tc:
        with (
            tc.tile_pool(name="const", bufs=1) as cpool,
            tc.tile_pool(name="qk", bufs=1) as qkpool,
            tc.tile_pool(name="vp", bufs=1) as vpool,
            tc.tile_pool(name="op", bufs=3) as opool,
            tc.tile_pool(name="rp", bufs=2) as rpool,
            tc.tile_pool(name="xw", bufs=1) as xwpool,
            tc.tile_pool(name="ex", bufs=17) as expool,
            tc.tile_pool(name="pss", bufs=3, space="PSUM") as pss,
            tc.tile_pool(name="psc", bufs=2, space="PSUM") as psc,
        ):
            actb = cpool.tile([128, 1], F32)
            nc.vector.memset(actb[:], ACT_BIAS)
            kexp = cpool.tile([128, 1], F32)
            nc.vector.memset(kexp[:], EXP_C3)

            def act_fill(ap_, value):
                # out = Copy(0 * 1 + value) on the Act engine; keeps GpSimd
                # unused so TileContext skips its expensive dge_drain
                zero = nc.const_aps.tensor(0.0, tuple(ap_.shape))
                nc.scalar.activation(
                    ap_, zero, mybir.ActivationFunctionType.Copy,
                    bias=float(value),
                )

            qT2 = qkpool.tile([128, HPC, S], BF16)
            kT2 = qkpool.tile([128, HPC, S], BF16)
            act_fill(qT2[64:65, :, :].rearrange("p h s -> p (h s)"), QROW)
            if use_mask:
                for h in range(HPC):
                    nc.sync.dma_start(kT2[64:65, h, :], kb_d[:])
            else:
                act_fill(kT2[64:65, :, :].rearrange("p h s -> p (h s)"), KROW)

            v8 = vpool.tile([128, NJ, HPC, VW], BF16)
            act_fill(v8[:, :, :, HEAD_DIM:VW], 1.0)

            xT = xwpool.tile([128, KC, S], BF16)
            wq = xwpool.tile([128, KC, HPC * HEAD_DIM], BF16)
            wk = xwpool.tile([128, KC, HPC * HEAD_DIM], BF16)
            wv = xwpool.tile([128, KC, HPC * HEAD_DIM], BF16)
            kc = KC if use_bias else KC - 1
            for c in range(kc):
                nc.sync.dma_start(xT[:, c, :], xT_d[ts(c, 128), :])
                nc.sync.dma_start(wq[:, c, :], wq_d[ts(c, 128), :])
                nc.sync.dma_start(wk[:, c, :], wk_d[ts(c, 128), :])
            for c in range(kc):
                nc.sync.dma_start(wv[:, c, :], wv_d[ts(c, 128), :])

            def emit_qk(p, which=(0, 1)):
                for w_, dst in [((wq, qT2), (wk, kT2))[w] for w in which]:
                    for half in range(2):
                        acc = pss.tile([128, 2, 512], F32, tag="s")
                        for c in range(kc):
                            for n2 in range(2):
                                n = 2 * half + n2
                                nc.tensor.matmul(
                                    acc[:, n2, :], w_[:, c, ts(p, 128)],
                                    xT[:, c, ts(n, 512)],
                                    start=(c == 0), stop=(c == kc - 1),
                                )
                        # head-even rows 0:64 stay aligned (Act engine);
                        # head-odd rows 64:128 shift down to 0:64 (DVE)
                        nc.scalar.copy(
                            dst[0:HEAD_DIM, 2 * p, ts(half, 1024)],
                            acc[0:HEAD_DIM, :, :].rearrange("p a n -> p (a n)"),
                        )
                        nc.vector.tensor_copy(
                            dst[0:HEAD_DIM, 2 * p + 1, ts(half, 1024)],
                            acc[HEAD_DIM:128, :, :].rearrange("p a n -> p (a n)"),
                        )

            def emit_v(jt):
                pv = pss.tile([128, HPC * HEAD_DIM], F32, tag="s")
                for c in range(kc):
                    nc.tensor.matmul(
                        pv[:], xT[:, c, ts(jt, 128)], wv[:, c, :],
                        start=(c == 0), stop=(c == kc - 1),
                    )
                nc.vector.tensor_copy(
                    v8[:, jt, :, 0:HEAD_DIM],
                    pv[:].rearrange("p (h e) -> p h e", h=HPC),
                )

            def emit_exp(eng, ss_t, ext, jslot):
                dst = ext[:, jslot, :, :]
                if eng == "a":
                    nc.scalar.activation(
                        dst, ss_t[:], mybir.ActivationFunctionType.Exp,
                        scale=ACT_SCALE, bias=actb[:, 0:1],
                    )
                else:
                    nc.vector._custom_dve(
                        EXP_BITS,
                        out=dst.rearrange("p a n -> p (a n)").bitcast(I16),
                        in0=ss_t[:].rearrange("p a n -> p (a n)"),
                        in1=kexp[:],
                        s0=MAGIC, s1=EXP_C1, imm2=EXP_C2,
                    )

            chunks = [(p_, i_) for p_ in range(HPC // 2) for i_ in range(NI)]

            emit_qk(0)

            prev = None  # ((pair, ic), ex_tiles)
            for ci in range(len(chunks) + 1):
                ch = chunks[ci] if ci < len(chunks) else None
                if prev is not None:
                    pc = [psc.tile([128, 512], F32, tag="c", name=f"pc{a}")
                          for a in range(2)]
                ex_tiles = []
                for jt in range(NJ if ch is not None else 0):
                    pr_, ic = ch
                    if jt % 2 == 0:
                        ext = expool.tile([128, 2, 2, 512], BF16, tag="e")
                        ex_tiles.append(ext)
                    ss_t = pss.tile([128, 2, 512], F32, tag="s")
                    for a_ in range(2):
                        h = 2 * pr_ + a_
                        nc.tensor.matmul(
                            ss_t[:, a_, :],
                            kT2[0:HEAD_DIM + 1, h, ts(jt, 128)],
                            qT2[0:HEAD_DIM + 1, h, ts(ic, 512)],
                            start=True, stop=True,
                        )
                    emit_exp(ENG[jt], ss_t, ext, jt % 2)
                    if ci == 0 and jt % 2 == 0:
                        emit_v(jt // 2)
                    if ci == 1 and jt % 2 == 0:
                        emit_v(8 + jt // 2)
                    if prev is not None:
                        for a_ in range(2):
                            h = 2 * prev[0][0] + a_
                            nc.tensor.matmul(
                                pc[a_][:],
                                v8[:, jt, h, :],
                                prev[1][jt // 2][:, jt % 2, a_, :],
                                start=(jt == 0), stop=(jt == NJ - 1),
                            )
                if ch is None and prev is not None:
                    # drain window: head-outer ctx so finalize of head a0
                    # overlaps head a1's matmul stream
                    for a_ in range(2):
                        h = 2 * prev[0][0] + a_
                        for jt in range(NJ):
                            nc.tensor.matmul(
                                pc[a_][:],
                                v8[:, jt, h, :],
                                prev[1][jt // 2][:, jt % 2, a_, :],
                                start=(jt == 0), stop=(jt == NJ - 1),
                            )
                if ci == 2:
                    emit_qk(1, (0,))
                elif ci == 3:
                    emit_qk(1, (1,))
                elif ci == 4:
                    emit_qk(2, (0,))
                elif ci == 5:
                    emit_qk(2, (1,))
                if prev is not None:
                    ppr, pic = prev[0]
                    for a_ in range(2):
                        h = 2 * ppr + a_
                        # move the 64 sumexp rows down to partitions 0:64
                        # (tensor_copy handles the partition shift) so the
                        # recip + mult stay partition-aligned
                        den = rpool.tile([64, 512], F32, tag="den")
                        nc.vector.tensor_copy(den[:], pc[a_][HEAD_DIM:VW, :])
                        rc = rpool.tile([64, 512], F32, tag="rc")
                        nc.vector.reciprocal_approx_fast(rc[:], den[:])
                        o = opool.tile([64, 512], F32, tag="o")
                        nc.vector.tensor_tensor(
                            o[:], pc[a_][0:HEAD_DIM, :], rc[:],
                            op=mybir.AluOpType.mult,
                        )
                        nc.sync.dma_start(out_d[h, :, ts(pic, 512)], o[:])
                prev = (ch, ex_tiles) if ch is not None else None

    nc.compile()
    return nc


def _get_nc(use_mask: bool, use_bias: bool):
    key = (use_mask, use_bias)
    if key not in _cache:
        _cache[key] = _build(use_mask, use_bias)
    return _cache[key]


def kernel(hidden_states, attention_mask, Wq, bq, Wk, bk, Wv, bv):
    global last_results
    hidden_states = np.asarray(hidden_states, dtype=np.float32)
    attention_mask = np.asarray(attention_mask, dtype=np.float32)
    Wq = np.asarray(Wq, dtype=np.float32)
    Wk = np.asarray(Wk, dtype=np.float32)
    Wv = np.asarray(Wv, dtype=np.float32)
    bq = np.asarray(bq, dtype=np.float32)
    bk = np.asarray(bk, dtype=np.float32)
    bv = np.asarray(bv, dtype=np.float32)

    use_mask = bool(np.any(attention_mask))
    use_bias = bool(np.any(bq) or np.any(bk) or np.any(bv))
    nc = _get_nc(use_mask, use_bias)

    bf16 = mybir.dt.np(BF16)
    QSCALE = 16.0 * float(np.log2(np.e))

    in_maps = []
    for c in range(8):
        b = c // 2
        hg = c % 2
        cs = slice(hg * HPC * HEAD_DIM, (hg + 1) * HPC * HEAD_DIM)

        xT = np.zeros((KC * 128, S), dtype=np.float32)
        xT[:HIDDEN] = hidden_states[b].T
        xT[HIDDEN] = 1.0

        def wslice(W, bias, scale=1.0):
            w = np.zeros((KC * 128, HPC * HEAD_DIM), dtype=np.float32)
            w[:HIDDEN] = W[:, cs]
            w[HIDDEN] = bias[cs]
            return (w * scale).astype(bf16)

        m = {
            "xT": xT.astype(bf16),
            "wq": wslice(Wq, bq, QSCALE),
            "wk": wslice(Wk, bk),
            "wv": wslice(Wv, bv),
        }
        if use_mask:
            mk = attention_mask[b, 0, 0, :].astype(np.float32)
            m["kb"] = ((ZBIAS + SCH_A * mk) / QROW).astype(bf16).reshape(1, S)
        in_maps.append(m)

    res = run_bass_kernel_spmd(
        nc, in_maps, list(range(8)),
        trace=bool(os.environ.get("KERNEL_TRACE")),
    )
    last_results = res

    out = np.empty((B, S, HIDDEN), dtype=np.float32)
    for c in range(8):
        b = c // 2
        hg = c % 2
        r = res.results[c]["out"]  # [6, 64, 2048]
        out[b, :, hg * HPC * HEAD_DIM:(hg + 1) * HPC * HEAD_DIM] = (
            r.transpose(2, 0, 1).reshape(S, HPC * HEAD_DIM)
        )
    return out
